# revision 2
# baseline (speedup 1.0000x reference)
"""Trainium2 Bass kernel for batched multi-head attention (8 NeuronCores).

Problem: x[8,1024,1024], Wq[1024,1024], bq[1024], Wkv[1024,2048], bkv[2048]
  q = x@Wq+bq ; k,v = split(x@Wkv+bkv, 2) ;
  out = softmax(q k^T / sqrt(64), over keys) @ v        (16 heads, d=64)

Sharding: data-parallel over batch — one batch element per NeuronCore;
weights replicated; no collectives. Outputs are stacked on the host.

Per-core kernel design (bf16 matmuls, fp32 PSUM accumulation):
  - x^T is pre-transposed on the host and DMA'd as plain contiguous chunks
    (the transposing-DMA path is slower and is unreliable on the
    Activation-engine queue).
  - q^T (c on partitions) is computed on the PE; bias fused into the
    PSUM->SBUF copy (per-partition bias add on VectorE).
  - k^T is stored PER HEAD in zero-padded [128, N] tiles (head h's 64 d-rows
    at their native partitions, the other 64 partitions zero).  The S=QK^T
    matmul then contracts K=128 instead of K=64: measured TRN2 PE throughput
    for K=64 matmuls is HALF rate (447ns vs 226ns per 512-row instruction),
    so padding with zeros doubles S throughput at zero numerics cost (the
    zero rows multiply the other head's q rows and contribute nothing).
  - s^T = kp.T @ q^T puts softmax keys on the PARTITION dim, so attention
    probabilities come out already transposed for the att@v matmul.
  - exp on ScalarE with the 1/sqrt(d) scale fused into the activation;
    max-subtraction is skipped (logits are bounded ~|4| by construction,
    exp is overflow-safe in fp32, and softmax is shift-invariant).
  - v carries an extra ones-column, so the att@v matmul emits the softmax
    denominator Z alongside the unnormalized output; normalization + v-bias
    are one fused VectorE op: out = (o * 1/Z) + bv.
  - Heads are software-pipelined: the att@v pass of head h-1 is interleaved
    between the S/exp pairs of head h, so the PE stays busy while ScalarE
    (the ~134us exp floor) drains the s tiles; the V projections fill the
    same slots during head 0.
  - Output is bf16 (host casts to fp32) and drains in 4-head column strips
    as soon as the strip's heads complete.
"""

from contextlib import ExitStack

import numpy as np
import ml_dtypes

import concourse.bass as bass
import concourse.mybir as mybir
import concourse.tile as tile
from concourse import bacc
from concourse.bass_utils import run_bass_kernel_spmd

P = 128
N = 1024
C = 1024
H = 16
D = 64
NCH = N // P
B = 8
SCALE = D ** -0.5
F32 = mybir.dt.float32
BF16 = mybir.dt.bfloat16
EXP = mybir.ActivationFunctionType.Exp
MULT = mybir.AluOpType.mult
ADD = mybir.AluOpType.add


def _build():
    nc = bacc.Bacc("TRN2")
    xt = nc.dram_tensor("xt", [C, N], BF16, kind="ExternalInput")
    wq = nc.dram_tensor("wq", [C, C], BF16, kind="ExternalInput")
    bq = nc.dram_tensor("bq", [C], F32, kind="ExternalInput")
    wkv = nc.dram_tensor("wkv", [C, 2 * C], BF16, kind="ExternalInput")
    bkv = nc.dram_tensor("bkv", [2 * C], F32, kind="ExternalInput")
    out = nc.dram_tensor("out", [N, C], BF16, kind="ExternalOutput")

    with ExitStack() as ctx:
        tc = ctx.enter_context(tile.TileContext(nc))
        persist = ctx.enter_context(tc.tile_pool(name="persist", bufs=1))

        xT_t = [persist.tile([P, N], BF16, tag=f"xT{i}", name=f"xT{i}")
                for i in range(NCH)]
        wq_t = [persist.tile([P, C], BF16, tag=f"wq{i}", name=f"wq{i}")
                for i in range(NCH)]
        wkv_t = [persist.tile([P, 2 * C], BF16, tag=f"wkv{i}", name=f"wkv{i}")
                 for i in range(NCH)]
        qT_t = [persist.tile([P, N], BF16, tag=f"qT{i}", name=f"qT{i}")
                for i in range(NCH)]
        kp_t = [persist.tile([P, N], BF16, tag=f"kp{i}", name=f"kp{i}")
                for i in range(H)]
        v_sb = persist.tile([P, NCH, H, D + 1], BF16, tag="v")
        out_t = [persist.tile([P, C], BF16, tag=f"out{i}", name=f"out{i}")
                 for i in range(NCH)]
        bq_sb = persist.tile([P, NCH], F32, tag="bq")
        bk_sb = persist.tile([P, NCH], F32, tag="bk")
        bv_bc = persist.tile([P, C], F32, tag="bv")
        scratch = persist.tile([P, 512], BF16, tag="scratch")

        pt_pool = ctx.enter_context(tc.tile_pool(name="pt", bufs=2))
        rz_pool = ctx.enter_context(tc.tile_pool(name="rz", bufs=8))
        proj_ps = ctx.enter_context(
            tc.tile_pool(name="proj_ps", bufs=2, space="PSUM"))
        s_ps = ctx.enter_context(tc.tile_pool(name="s_ps", bufs=2, space="PSUM"))
        o_ps = ctx.enter_context(tc.tile_pool(name="o_ps", bufs=2, space="PSUM"))

        # PE warmup (p-state ramp) on a zeroed scratch tile; scratch is
        # zeroed on ScalarE so the warmup chain starts immediately while
        # VectorE zero-fills the padded kp tiles.
        nc.scalar.memzero(scratch[:])
        for h in range(H):
            eng = nc.vector if h % 2 == 0 else nc.gpsimd
            eng.memset(kp_t[h][:], 0.0)
        nc.vector.memset(v_sb[:, :, :, D], 1.0)
        wps = proj_ps.tile([P, 512], F32, tag="ps", name="wups")
        for _ in range(10):
            nc.tensor.matmul(wps[:], scratch[:, 0:P], scratch[:],
                             start=True, stop=True)

        # cold input DMAs, first-needed-first: biases, x chunks, then
        # weights in first-use order; round-robin over three queues
        DQ3 = [nc.sync, nc.scalar, nc.gpsimd]
        qi = 0

        def cold(dma_out, dma_in):
            nonlocal qi
            DQ3[qi % 3].dma_start(out=dma_out, in_=dma_in)
            qi += 1

        nc.sync.dma_start(out=bk_sb[:],
                          in_=bkv.ap()[0:C].rearrange("(cc p) -> p cc", p=P))
        nc.scalar.dma_start(out=bq_sb[:],
                            in_=bq.ap().rearrange("(cc p) -> p cc", p=P))
        bv_row = bkv.ap()[C:2 * C]
        nc.sync.dma_start(
            out=bv_bc[:],
            in_=bass.AP(tensor=bv_row.tensor, offset=bv_row.offset,
                        ap=[[0, P]] + list(bv_row.ap)),
        )
        for cc in range(NCH):
            cold(xT_t[cc][:], xt.ap()[cc * P:(cc + 1) * P, :])
        for kc in range(NCH):
            cold(wkv_t[kc][:, 0:C], wkv.ap()[kc * P:(kc + 1) * P, 0:C])
        for kc in range(NCH):
            cold(wq_t[kc][:], wq.ap()[kc * P:(kc + 1) * P, :])
        for kc in range(NCH):
            cold(wkv_t[kc][:, C:2 * C],
                 wkv.ap()[kc * P:(kc + 1) * P, C:2 * C])

        def proj_group(lhs_fn, rhs_fn):
            ps = proj_ps.tile([P, 512], F32, tag="ps", name="ps")
            for kc in range(NCH):
                nc.tensor.matmul(ps[:], lhs_fn(kc), rhs_fn(kc),
                                 start=(kc == 0), stop=(kc == NCH - 1))
            return ps

        def v_proj(mc, ch):
            ps = proj_group(
                lambda kc: xT_t[kc][:, mc * P:(mc + 1) * P],
                lambda kc: wkv_t[kc][:, C + ch * 512:C + (ch + 1) * 512],
            )
            nc.vector.tensor_copy(
                v_sb[:, mc, ch * 8:(ch + 1) * 8, 0:D],
                ps[:].rearrange("p (h d) -> p h d", d=D),
            )

        def emit_o(h, pt, ni):
            o = o_ps.tile([P, D + 1], F32, tag="o", name="o")
            for mc in range(NCH):
                nc.tensor.matmul(
                    o[:], pt[:, mc, ni * P:(ni + 1) * P],
                    v_sb[:, mc, h, :],
                    start=(mc == 0), stop=(mc == NCH - 1))
            rz = rz_pool.tile([P, 1], F32, tag="rz", name="rz")
            nc.vector.reciprocal(rz[:], o[:, D:D + 1])
            nc.vector.scalar_tensor_tensor(
                out_t[ni][:, h * D:(h + 1) * D],
                o[:, 0:D], rz[:], bv_bc[:, h * D:(h + 1) * D],
                op0=MULT, op1=ADD)

        prev = None
        for cc in range(NCH):
            for nh in range(2):
                ps = proj_group(
                    lambda kc: wkv_t[kc][:, cc * P:(cc + 1) * P],
                    lambda kc: xT_t[kc][:, nh * 512:(nh + 1) * 512],
                )
                # one PSUM->SBUF copy per padded head tile, partition ranges
                # preserved (rows 0:64 -> even head, 64:128 -> odd head)
                nc.vector.tensor_scalar_add(
                    kp_t[2 * cc][0:D, nh * 512:(nh + 1) * 512],
                    ps[0:D, :], bk_sb[0:D, cc:cc + 1])
                nc.vector.tensor_scalar_add(
                    kp_t[2 * cc + 1][D:2 * D, nh * 512:(nh + 1) * 512],
                    ps[D:2 * D, :], bk_sb[D:2 * D, cc:cc + 1])
            for nh in range(2):
                ps = proj_group(
                    lambda kc: wq_t[kc][:, cc * P:(cc + 1) * P],
                    lambda kc: xT_t[kc][:, nh * 512:(nh + 1) * 512],
                )
                nc.vector.tensor_scalar_add(
                    qT_t[cc][:, nh * 512:(nh + 1) * 512], ps[:],
                    bq_sb[:, cc:cc + 1])

            for hr in range(2):
                h = 2 * cc + hr
                pt = pt_pool.tile([P, NCH, N], BF16, tag="pt", name="pt")
                for mc in range(NCH):
                    s = s_ps.tile([P, N], F32, tag="s", name="s")
                    for nh in range(2):
                        nc.tensor.matmul(
                            s[:, nh * 512:(nh + 1) * 512],
                            kp_t[h][:, mc * P:(mc + 1) * P],
                            qT_t[cc][:, nh * 512:(nh + 1) * 512],
                            start=True, stop=True)
                    nc.scalar.activation(pt[:, mc, :], s[:], EXP, scale=SCALE)
                    if h == 0:
                        v_proj(mc, 0)
                        v_proj(mc, 1)
                    elif prev is not None:
                        emit_o(prev[0], prev[1], mc)
                prev = (h, pt)
                if h in (4, 8, 12):
                    hb = h - 4
                    for ni in range(NCH):
                        eng = nc.sync if ni % 2 == 0 else nc.scalar
                        eng.dma_start(
                            out=out.ap()[ni * P:(ni + 1) * P,
                                         hb * D:(hb + 4) * D],
                            in_=out_t[ni][:, hb * D:(hb + 4) * D])
        for ni in range(NCH):
            emit_o(prev[0], prev[1], ni)
        for ni in range(NCH):
            eng = nc.sync if ni % 2 == 0 else nc.scalar
            eng.dma_start(out=out.ap()[ni * P:(ni + 1) * P, 12 * D:],
                          in_=out_t[ni][:, 12 * D:])

    nc.finalize()
    return nc


_NC = None


def kernel(x, Wq, bq, Wkv, bkv):
    global _NC
    if _NC is None:
        _NC = _build()
    bf = ml_dtypes.bfloat16
    x = np.asarray(x)
    wq_b = np.asarray(Wq).astype(bf)
    wkv_b = np.asarray(Wkv).astype(bf)
    bq_f = np.asarray(bq).astype(np.float32)
    bkv_f = np.asarray(bkv).astype(np.float32)
    in_maps = [
        {"xt": np.ascontiguousarray(x[b].astype(bf).T), "wq": wq_b,
         "bq": bq_f, "wkv": wkv_b, "bkv": bkv_f}
        for b in range(B)
    ]
    res = run_bass_kernel_spmd(_NC, in_maps, core_ids=list(range(B)))
    return np.stack([res.results[b]["out"] for b in range(B)]).astype(np.float32)


# revision 3
# speedup vs baseline: 2.1933x; 2.1933x over previous
"""Trainium2 Bass kernel for batched multi-head attention (8 NeuronCores).

Problem: x[8,1024,1024], Wq[1024,1024], bq[1024], Wkv[1024,2048], bkv[2048]
  q = x@Wq+bq ; k,v = split(x@Wkv+bkv, 2) ;
  out = softmax(q k^T / sqrt(64), over keys) @ v        (16 heads, d=64)

Sharding: data-parallel over batch — one batch element per NeuronCore;
weights replicated; no collectives. Outputs are stacked on the host.

Per-core kernel design (bf16 matmuls, fp32 PSUM accumulation):
  - x^T is pre-transposed on the host and DMA'd as plain contiguous chunks
    (the transposing-DMA path is slower and is unreliable on the
    Activation-engine queue).
  - q^T (c on partitions) is computed on the PE; bias fused into the
    PSUM->SBUF copy (per-partition bias add on VectorE).
  - k^T is stored PER HEAD in zero-padded [128, N] tiles (head h's 64 d-rows
    at their native partitions, the other 64 partitions zero).  The S=QK^T
    matmul then contracts K=128 instead of K=64: measured TRN2 PE throughput
    for K=64 matmuls is HALF rate (447ns vs 226ns per 512-row instruction),
    so padding with zeros doubles S throughput at zero numerics cost (the
    zero rows multiply the other head's q rows and contribute nothing).
  - s^T = kp.T @ q^T puts softmax keys on the PARTITION dim, so attention
    probabilities come out already transposed for the att@v matmul.
  - exp on ScalarE with the 1/sqrt(d) scale fused into the activation;
    max-subtraction is skipped (logits are bounded ~|4| by construction,
    exp is overflow-safe in fp32, and softmax is shift-invariant).
  - v carries an extra ones-column, so the att@v matmul emits the softmax
    denominator Z alongside the unnormalized output; normalization + v-bias
    are one fused VectorE op: out = (o * 1/Z) + bv.
  - Heads are software-pipelined: the att@v pass of head h-1 is interleaved
    between the S/exp pairs of head h, so the PE stays busy while ScalarE
    (the ~134us exp floor) drains the s tiles; the V projections fill the
    same slots during head 0.
  - Output is bf16 (host casts to fp32) and drains in 4-head column strips
    as soon as the strip's heads complete.
"""

from contextlib import ExitStack

import numpy as np
import ml_dtypes

import concourse.bass as bass
import concourse.mybir as mybir
import concourse.tile as tile
from concourse import bacc
from concourse.bass_utils import run_bass_kernel_spmd

P = 128
N = 1024
C = 1024
H = 16
D = 64
NCH = N // P
B = 8
SCALE = D ** -0.5
F32 = mybir.dt.float32
BF16 = mybir.dt.bfloat16
EXP = mybir.ActivationFunctionType.Exp
MULT = mybir.AluOpType.mult
ADD = mybir.AluOpType.add


def _build():
    nc = bacc.Bacc("TRN2")
    xt = nc.dram_tensor("xt", [C, N], BF16, kind="ExternalInput")
    wq = nc.dram_tensor("wq", [C, C], BF16, kind="ExternalInput")
    bq = nc.dram_tensor("bq", [C], F32, kind="ExternalInput")
    wkv = nc.dram_tensor("wkv", [C, 2 * C], BF16, kind="ExternalInput")
    bkv = nc.dram_tensor("bkv", [2 * C], F32, kind="ExternalInput")
    out = nc.dram_tensor("out", [N, C], BF16, kind="ExternalOutput")

    with ExitStack() as ctx:
        tc = ctx.enter_context(tile.TileContext(nc))
        persist = ctx.enter_context(tc.tile_pool(name="persist", bufs=1))

        xT_t = [persist.tile([P, N], BF16, tag=f"xT{i}", name=f"xT{i}")
                for i in range(NCH)]
        wq_t = [persist.tile([P, C], BF16, tag=f"wq{i}", name=f"wq{i}")
                for i in range(NCH)]
        wkv_t = [persist.tile([P, 2 * C], BF16, tag=f"wkv{i}", name=f"wkv{i}")
                 for i in range(NCH)]
        qT_t = [persist.tile([P, N], BF16, tag=f"qT{i}", name=f"qT{i}")
                for i in range(NCH)]
        kp_t = [persist.tile([P, N], BF16, tag=f"kp{i}", name=f"kp{i}")
                for i in range(H)]
        v_sb = persist.tile([P, NCH, H, D + 1], BF16, tag="v")
        out_t = [persist.tile([P, C], BF16, tag=f"out{i}", name=f"out{i}")
                 for i in range(NCH)]
        bq_sb = persist.tile([P, NCH], F32, tag="bq")
        bk_sb = persist.tile([P, NCH], F32, tag="bk")
        bv_bc = persist.tile([P, C], F32, tag="bv")
        scratch = persist.tile([P, 512], BF16, tag="scratch")

        pt_pool = ctx.enter_context(tc.tile_pool(name="pt", bufs=2))
        rz_pool = ctx.enter_context(tc.tile_pool(name="rz", bufs=8))
        proj_ps = ctx.enter_context(
            tc.tile_pool(name="proj_ps", bufs=2, space="PSUM"))
        s_ps = ctx.enter_context(tc.tile_pool(name="s_ps", bufs=2, space="PSUM"))
        o_ps = ctx.enter_context(tc.tile_pool(name="o_ps", bufs=2, space="PSUM"))

        # PE warmup (p-state ramp) on a zeroed scratch tile; scratch is
        # zeroed on ScalarE so the warmup chain starts immediately while
        # VectorE zero-fills the padded kp tiles.
        nc.scalar.memzero(scratch[:])
        for h in range(H):
            eng = nc.vector if h % 2 == 0 else nc.gpsimd
            eng.memset(kp_t[h][:], 0.0)
        nc.vector.memset(v_sb[:, :, :, D], 1.0)
        wps = proj_ps.tile([P, 512], F32, tag="ps", name="wups")
        for _ in range(10):
            nc.tensor.matmul(wps[:], scratch[:, 0:P], scratch[:],
                             start=True, stop=True)

        # cold input DMAs, first-needed-first: biases, x chunks, then
        # weights in first-use order; round-robin over three queues
        DQ3 = [nc.sync, nc.scalar, nc.gpsimd]
        qi = 0

        def cold(dma_out, dma_in):
            nonlocal qi
            DQ3[qi % 3].dma_start(out=dma_out, in_=dma_in)
            qi += 1

        nc.sync.dma_start(out=bk_sb[:],
                          in_=bkv.ap()[0:C].rearrange("(cc p) -> p cc", p=P))
        nc.scalar.dma_start(out=bq_sb[:],
                            in_=bq.ap().rearrange("(cc p) -> p cc", p=P))
        bv_row = bkv.ap()[C:2 * C]
        nc.sync.dma_start(
            out=bv_bc[:],
            in_=bass.AP(tensor=bv_row.tensor, offset=bv_row.offset,
                        ap=[[0, P]] + list(bv_row.ap)),
        )
        for cc in range(NCH):
            cold(xT_t[cc][:], xt.ap()[cc * P:(cc + 1) * P, :])
        for kc in range(NCH):
            cold(wkv_t[kc][:, 0:C], wkv.ap()[kc * P:(kc + 1) * P, 0:C])
        for kc in range(NCH):
            cold(wq_t[kc][:], wq.ap()[kc * P:(kc + 1) * P, :])
        for kc in range(NCH):
            cold(wkv_t[kc][:, C:2 * C],
                 wkv.ap()[kc * P:(kc + 1) * P, C:2 * C])

        def proj_group(lhs_fn, rhs_fn):
            ps = proj_ps.tile([P, 512], F32, tag="ps", name="ps")
            for kc in range(NCH):
                nc.tensor.matmul(ps[:], lhs_fn(kc), rhs_fn(kc),
                                 start=(kc == 0), stop=(kc == NCH - 1))
            return ps

        def v_proj(mc, ch):
            ps = proj_group(
                lambda kc: xT_t[kc][:, mc * P:(mc + 1) * P],
                lambda kc: wkv_t[kc][:, C + ch * 512:C + (ch + 1) * 512],
            )
            nc.vector.tensor_copy(
                v_sb[:, mc, ch * 8:(ch + 1) * 8, 0:D],
                ps[:].rearrange("p (h d) -> p h d", d=D),
            )

        def emit_o(h, pt, ni):
            o = o_ps.tile([P, D + 1], F32, tag="o", name="o")
            for mc in range(NCH):
                nc.tensor.matmul(
                    o[:], pt[:, mc, ni * P:(ni + 1) * P],
                    v_sb[:, mc, h, :],
                    start=(mc == 0), stop=(mc == NCH - 1))
            rz = rz_pool.tile([P, 1], F32, tag="rz", name="rz")
            nc.vector.reciprocal(rz[:], o[:, D:D + 1])
            nc.vector.scalar_tensor_tensor(
                out_t[ni][:, h * D:(h + 1) * D],
                o[:, 0:D], rz[:], bv_bc[:, h * D:(h + 1) * D],
                op0=MULT, op1=ADD)

        prev = None
        for cc in range(NCH):
            for nh in range(2):
                ps = proj_group(
                    lambda kc: wkv_t[kc][:, cc * P:(cc + 1) * P],
                    lambda kc: xT_t[kc][:, nh * 512:(nh + 1) * 512],
                )
                # one PSUM->SBUF copy per padded head tile, partition ranges
                # preserved (rows 0:64 -> even head, 64:128 -> odd head)
                nc.vector.tensor_scalar_add(
                    kp_t[2 * cc][0:D, nh * 512:(nh + 1) * 512],
                    ps[0:D, :], bk_sb[0:D, cc:cc + 1])
                nc.vector.tensor_scalar_add(
                    kp_t[2 * cc + 1][D:2 * D, nh * 512:(nh + 1) * 512],
                    ps[D:2 * D, :], bk_sb[D:2 * D, cc:cc + 1])
            for nh in range(2):
                ps = proj_group(
                    lambda kc: wq_t[kc][:, cc * P:(cc + 1) * P],
                    lambda kc: xT_t[kc][:, nh * 512:(nh + 1) * 512],
                )
                nc.vector.tensor_scalar_add(
                    qT_t[cc][:, nh * 512:(nh + 1) * 512], ps[:],
                    bq_sb[:, cc:cc + 1])

            for hr in range(2):
                h = 2 * cc + hr
                pt = pt_pool.tile([P, NCH, N], BF16, tag="pt", name="pt")
                for mc in range(NCH):
                    s = s_ps.tile([P, N], F32, tag="s", name="s")
                    for nh in range(2):
                        nc.tensor.matmul(
                            s[:, nh * 512:(nh + 1) * 512],
                            kp_t[h][:, mc * P:(mc + 1) * P],
                            qT_t[cc][:, nh * 512:(nh + 1) * 512],
                            start=True, stop=True)
                    nc.scalar.activation(pt[:, mc, :], s[:], EXP, scale=SCALE)
                    if h == 0:
                        v_proj(mc, 0)
                        v_proj(mc, 1)
                    elif prev is not None:
                        emit_o(prev[0], prev[1], mc)
                prev = (h, pt)
                if h in (4, 8, 12):
                    hb = h - 4
                    for ni in range(NCH):
                        eng = nc.sync if ni % 2 == 0 else nc.scalar
                        eng.dma_start(
                            out=out.ap()[ni * P:(ni + 1) * P,
                                         hb * D:(hb + 4) * D],
                            in_=out_t[ni][:, hb * D:(hb + 4) * D])
        for ni in range(NCH):
            emit_o(prev[0], prev[1], ni)
        for ni in range(NCH):
            eng = nc.sync if ni % 2 == 0 else nc.scalar
            eng.dma_start(out=out.ap()[ni * P:(ni + 1) * P, 12 * D:],
                          in_=out_t[ni][:, 12 * D:])

    nc.finalize()
    return nc


_NC = None
_FAST = None  # (jitted_fn, input_names, out_shape_dtype) — axon/PJRT path


def _make_fast(nc):
    """Persistent jitted PJRT executor (axon path): build the shard_map once
    so repeated kernel() calls skip jax re-tracing/compilation."""
    import jax
    from jax.sharding import Mesh, PartitionSpec
    from jax.experimental.shard_map import shard_map
    from concourse import bass2jax
    from concourse.bass2jax import _bass_exec_p, partition_id_tensor

    bass2jax.install_neuronx_cc_hook()
    partition_name = (nc.partition_id_tensor.name
                      if nc.partition_id_tensor else None)
    in_names, out_names, out_avals, zero_outs = [], [], [], []
    for alloc in nc.m.functions[0].allocations:
        if not isinstance(alloc, mybir.MemoryLocationSet):
            continue
        name = alloc.memorylocations[0].name
        if alloc.kind == "ExternalInput":
            if name != partition_name:
                in_names.append(name)
        elif alloc.kind == "ExternalOutput":
            shape = tuple(alloc.tensor_shape)
            dtype = mybir.dt.np(alloc.dtype)
            out_names.append(name)
            out_avals.append(jax.core.ShapedArray(shape, dtype))
            zero_outs.append(np.zeros(shape, dtype))
    n_params = len(in_names)
    all_in = list(in_names) + list(out_names)
    if partition_name is not None:
        all_in.append(partition_name)

    def _body(*args):
        operands = list(args)
        if partition_name is not None:
            operands.append(partition_id_tensor())
        return tuple(_bass_exec_p.bind(
            *operands,
            out_avals=tuple(out_avals),
            in_names=tuple(all_in),
            out_names=tuple(out_names),
            lowering_input_output_aliases=(),
            sim_require_finite=False,
            sim_require_nnan=False,
            nc=nc,
        ))

    devices = jax.devices()[:B]
    assert len(devices) == B
    mesh = Mesh(np.asarray(devices), ("core",))
    specs_in = (PartitionSpec("core"),) * (n_params + len(out_names))
    specs_out = (PartitionSpec("core"),) * len(out_names)
    fn = jax.jit(shard_map(_body, mesh=mesh, in_specs=specs_in,
                           out_specs=specs_out, check_rep=False),
                 keep_unused=True)
    return fn, in_names, out_names, out_avals, zero_outs


def kernel(x, Wq, bq, Wkv, bkv):
    global _NC, _FAST
    if _NC is None:
        _NC = _build()
    bf = ml_dtypes.bfloat16
    x = np.asarray(x)
    wq_b = np.asarray(Wq).astype(bf)
    wkv_b = np.asarray(Wkv).astype(bf)
    bq_f = np.asarray(bq).astype(np.float32)
    bkv_f = np.asarray(bkv).astype(np.float32)
    in_maps = [
        {"xt": np.ascontiguousarray(x[b].astype(bf).T), "wq": wq_b,
         "bq": bq_f, "wkv": wkv_b, "bkv": bkv_f}
        for b in range(B)
    ]
    try:
        from concourse._compat import axon_active
        if not axon_active():
            raise RuntimeError("native path")
        if _FAST is None:
            _FAST = _make_fast(_NC)
        fn, in_names, out_names, out_avals, zero_outs = _FAST
        concat_in = [np.concatenate([in_maps[c][n] for c in range(B)], axis=0)
                     for n in in_names]
        concat_zero = [np.zeros((B * z.shape[0], *z.shape[1:]), z.dtype)
                       for z in zero_outs]
        outs = fn(*concat_in, *concat_zero)
        oi = out_names.index("out")
        full = np.asarray(outs[oi]).reshape(B, *out_avals[oi].shape)
        return full.astype(np.float32)
    except Exception:
        res = run_bass_kernel_spmd(_NC, in_maps, core_ids=list(range(B)))
        return np.stack([res.results[b]["out"]
                         for b in range(B)]).astype(np.float32)
